# revision 25
# baseline (speedup 1.0000x reference)
"""Trainium2 Bass kernel for nn_EncoderRNN (batched GRU-step encoder).

Math: the reference is
    emb = x @ w_emb.T + b_emb
    gi  = emb @ w_ih.T + b_ih
    r   = sigmoid(gi_r + b_hr); z = sigmoid(gi_z + b_hz)
    n   = tanh(gi_n + r * b_hn)
    h   = (1 - z) * n
Both matmuls are linear, so they fold into one K=128 contraction:
    gi = x @ W.T + bias,  W = w_ih @ w_emb,  bias = w_ih @ b_emb + b_ih.
1 - sigmoid(a) = sigmoid(-a), so the z block of W/bias is negated on the
host and the device computes
    h = sigmoid(gi_zneg) * tanh(gi_n + sigmoid(gi_r) * b_hn).

fp32 matmuls on trn2 run as two half-rate passes (~5x slower than 16-bit),
so the product is computed in fp16 with fp32 PSUM accumulation:
    x @ W ~= xh@Wh + xl@Wh                  (x = xh + xl, both fp16)
The only dropped term is x @ (W - fp16(W)) ~ 2^-11 relative.  The bias
rides inside the xl matmul: two x-channels of xl are set to 1.0 and that
term's private copy of Wh carries the fp16 hi/lo bias rows there (the two
channels are chosen to minimize the absmax of the dropped xl correction).
Using 16-bit operands also lets x be loaded pre-transposed via the 2-byte
DMA x-bar transpose (the matmul contracts over i, which must sit on SBUF
partitions), so no compute engine spends time transposing.

Per 128-token tile: 6 matmuls (2 per gate bank), one sigmoid over the
r/z PSUM bank pair (ACT), r*b_hn and + n-bank on DVE, tanh on ACT, and
the final (1-z)*n product on GPSIMD; tanh/h-mul are software-pipelined
one tile behind so the strict-FIFO ACT queue never delays the sigmoid
whose completion releases the r/z PSUM banks.

Distribution: pure data parallel over the batch dim, 8 NeuronCores,
16 batches (8192 tokens) per core.  Weights are replicated.
"""

import numpy as np
import ml_dtypes

B, S, I, Hd = 128, 512, 128, 512
G3 = 3 * Hd
N_CORES = 8
B_PER_CORE = B // N_CORES            # 16
TOK = B_PER_CORE * S                 # 8192 tokens per core
GROUP_TOK = 512                      # tokens per group (4 tiles of 128)
J = GROUP_TOK // 128                 # 4 tiles per group
N_GROUPS = TOK // GROUP_TOK          # 16 groups per core

F16 = np.float16

_compiled = {}


def _build_program():
    import concourse.bacc as bacc
    import concourse.tile as tile
    from concourse import mybir

    F32 = mybir.dt.float32
    HF = mybir.dt.float16
    AF = mybir.ActivationFunctionType
    ALU = mybir.AluOpType

    nc = bacc.Bacc()
    xh_p = nc.declare_dram_parameter("xh", [TOK, I], HF, isOutput=False)
    xl_p = nc.declare_dram_parameter("xl", [TOK, I], HF, isOutput=False)
    wh_p = nc.declare_dram_parameter("wh", [I, G3], HF, isOutput=False)
    wha_p = nc.declare_dram_parameter("wha", [I, G3], HF, isOutput=False)
    bhn_p = nc.declare_dram_parameter("bhn", [Hd], F32, isOutput=False)
    out_p = nc.declare_dram_parameter("out", [TOK, Hd], F32, isOutput=True)

    out_v = out_p.rearrange("(g j p) h -> g p j h", p=128, j=J)

    with tile.TileContext(nc) as tc:
        with (
            tc.tile_pool(name="const", bufs=1) as cpool,
            tc.tile_pool(name="xin", bufs=8) as xin_pool,
            tc.tile_pool(name="ps", bufs=2, space="PSUM") as ps_pool,
            tc.tile_pool(name="work", bufs=8) as wpool,
            tc.tile_pool(name="hout", bufs=6) as hpool,
        ):
            # first group's transposed x loads go ahead of the const DMAs:
            # everything shares the SP HWDGE FIFO and the first matmul
            # needs the transposes.
            xht0 = xin_pool.tile([I, GROUP_TOK], HF, tag="xht")
            nc.sync.dma_start(out=xht0, in_=xh_p[0:GROUP_TOK, :],
                              transpose=True)
            xlt0 = xin_pool.tile([I, GROUP_TOK], HF, tag="xlt")
            nc.sync.dma_start(out=xlt0, in_=xl_p[0:GROUP_TOK, :],
                              transpose=True)
            wh_sb = cpool.tile([I, G3], HF)
            nc.sync.dma_start(out=wh_sb, in_=wh_p[:])
            wha_sb = cpool.tile([I, G3], HF)
            nc.sync.dma_start(out=wha_sb, in_=wha_p[:])
            bhn_sb = cpool.tile([128, Hd], F32)
            nc.gpsimd.dma_start(out=bhn_sb,
                                in_=bhn_p[:].partition_broadcast(128))

            # Software pipeline: the tanh + h-mul of tile t run one tile
            # late, so on the strict-FIFO ACT queue sigma(t+1) is never
            # stuck behind tanh(t) (whose input arrives via DVE) and the
            # r/z PSUM banks release as early as possible.
            NT = N_GROUPS * J
            h_gs = {}
            rzs = {}
            t2p = {}

            def head(t):
                g, j = divmod(t, J)
                if j == 0:
                    if g == 0:
                        xht, xlt = xht0, xlt0
                    else:
                        t0 = g * GROUP_TOK
                        xht = xin_pool.tile([I, GROUP_TOK], HF, tag="xht")
                        nc.sync.dma_start(
                            out=xht, in_=xh_p[t0:t0 + GROUP_TOK, :],
                            transpose=True
                        )
                        xlt = xin_pool.tile([I, GROUP_TOK], HF, tag="xlt")
                        nc.sync.dma_start(
                            out=xlt, in_=xl_p[t0:t0 + GROUP_TOK, :],
                            transpose=True
                        )
                    h_gs[g] = hpool.tile([128, J, Hd], F32, name="h_g",
                                         tag="h_g")
                    head.x = (xht, xlt)
                xht, xlt = head.x
                xh_j = xht[:, j * 128:(j + 1) * 128]
                xl_j = xlt[:, j * 128:(j + 1) * 128]
                # r/z banks in one 2-bank tile that frees right after the
                # sigmoid; the n bank separate (it is consumed last, so
                # deeper buffering keeps the PE from stalling on it).
                rz_ps = ps_pool.tile([128, 2 * Hd], F32, tag="rz_ps")
                n_ps = ps_pool.tile([128, Hd], F32, tag="n_ps", bufs=4)
                for b in range(2):
                    s = slice(b * Hd, (b + 1) * Hd)
                    nc.tensor.matmul(rz_ps[:, s], lhsT=xl_j,
                                     rhs=wha_sb[:, s], start=True, stop=False)
                    nc.tensor.matmul(rz_ps[:, s], lhsT=xh_j,
                                     rhs=wh_sb[:, s], start=False, stop=True)
                # r and z' share one sigmoid over two adjacent PSUM banks
                rz = wpool.tile([128, 2 * Hd], F32, tag="rz", bufs=6)
                nc.scalar.activation(rz, rz_ps, AF.Sigmoid)
                rzs[t] = rz
                s = slice(2 * Hd, G3)
                nc.tensor.matmul(n_ps, lhsT=xl_j, rhs=wha_sb[:, s],
                                 start=True, stop=False)
                nc.tensor.matmul(n_ps, lhsT=xh_j, rhs=wh_sb[:, s],
                                 start=False, stop=True)
                t1 = wpool.tile([128, Hd], F32, tag="t1", bufs=6)
                nc.vector.tensor_tensor(
                    out=t1, in0=rz[:, 0:Hd], in1=bhn_sb, op=ALU.mult
                )
                k, half = divmod(t, 2)
                if half == 0:
                    t2p[k] = wpool.tile([128, 2, Hd], F32, name="t2p",
                                        tag="t2p", bufs=4)
                nc.vector.tensor_tensor(out=t2p[k][:, half, :], in0=t1,
                                        in1=n_ps, op=ALU.add)

            def tailpair(k):
                # one tanh covers two tiles' n-gate inputs
                nn_p = wpool.tile([128, 2, Hd], F32, name="nn_p", tag="nn_p",
                                  bufs=4)
                nc.scalar.activation(nn_p, t2p.pop(k), AF.Tanh)
                for half in (0, 1):
                    t = 2 * k + half
                    g, j = divmod(t, J)
                    rz = rzs.pop(t)
                    nc.gpsimd.tensor_tensor(
                        out=h_gs[g][:, j, :], in0=rz[:, Hd:2 * Hd],
                        in1=nn_p[:, half, :], op=ALU.mult,
                    )
                    if j == J - 1:
                        nc.sync.dma_start(out=out_v[g], in_=h_gs.pop(g))

            for t in range(NT):
                head(t)
                if t % 2 == 1:
                    tailpair(t // 2)

    nc.finalize()
    return nc


def _split_f16(a):
    hi = a.astype(F16)
    lo = (a.astype(np.float64) - hi.astype(np.float64)).astype(F16)
    return hi, lo


def _prepare_consts(w_emb, b_emb, w_ih, b_ih, b_hh):
    # Fold the two linear layers (double precision for the fold itself).
    W = w_ih.astype(np.float64) @ w_emb.astype(np.float64)          # [3Hd, I]
    bias = w_ih.astype(np.float64) @ b_emb.astype(np.float64) + b_ih  # [3Hd]
    b_hr, b_hz, b_hn = b_hh[:Hd], b_hh[Hd:2 * Hd], b_hh[2 * Hd:]
    bias = bias.copy()
    bias[0:Hd] += b_hr
    bias[Hd:2 * Hd] += b_hz
    # 1 - sigmoid(a) = sigmoid(-a): negate the z block of W and bias.
    W[Hd:2 * Hd, :] *= -1.0
    bias[Hd:2 * Hd] *= -1.0
    wh = np.ascontiguousarray(W.T).astype(F16)                      # [I, 3Hd]
    bh, bl = _split_f16(bias)
    bhn = np.ascontiguousarray(b_hn).astype(np.float32)
    return wh, bh, bl, bhn


def _run(x, wh, bh, bl, bhn, trace=False):
    from concourse.bass_utils import run_bass_kernel_spmd

    if "nc" not in _compiled:
        _compiled["nc"] = _build_program()
    nc = _compiled["nc"]

    xh, xl = _split_f16(np.asarray(x, dtype=np.float32))
    xh = xh.reshape(N_CORES * TOK, I)
    xl = xl.reshape(N_CORES * TOK, I)
    # The bias rows ride inside the xl matmul: two channels of xl become
    # the constant 1.0 and that term's private copy of wh gets the bias
    # hi/lo rows there.  Pick the two channels that minimize the absmax
    # of the dropped xl-correction.
    p = np.abs(xl.astype(np.float32)).max(axis=0) * \
        np.abs(wh.astype(np.float32)).max(axis=1)
    c1, c2 = map(int, np.argsort(p)[:2])
    wha = wh.copy()
    wha[c1, :] = bh
    wha[c2, :] = bl
    xl = xl.copy()
    xl[:, c1] = 1.0
    xl[:, c2] = 1.0
    xh = xh.reshape(N_CORES, TOK, I)
    xl = xl.reshape(N_CORES, TOK, I)
    in_maps = [
        {"xh": xh[c], "xl": xl[c], "wh": wh, "wha": wha, "bhn": bhn}
        for c in range(N_CORES)
    ]
    res = run_bass_kernel_spmd(nc, in_maps, list(range(N_CORES)), trace=trace)
    full = np.stack([res.results[c]["out"] for c in range(N_CORES)], axis=0)
    full = full.reshape(B, S, Hd)
    return full, res


def kernel(x, w_emb, b_emb, w_ih, b_ih, b_hh):
    x = np.asarray(x, dtype=np.float32)
    consts = _prepare_consts(
        np.asarray(w_emb), np.asarray(b_emb), np.asarray(w_ih),
        np.asarray(b_ih), np.asarray(b_hh),
    )
    full, _ = _run(x, *consts, trace=False)
    H = np.ascontiguousarray(full[:, :-1, :])
    h_last = np.ascontiguousarray(full[:, -1, :][None])
    return (H, h_last)


# revision 26
# speedup vs baseline: 1.0530x; 1.0530x over previous
"""Trainium2 Bass kernel for nn_EncoderRNN (batched GRU-step encoder).

Math: the reference is
    emb = x @ w_emb.T + b_emb
    gi  = emb @ w_ih.T + b_ih
    r   = sigmoid(gi_r + b_hr); z = sigmoid(gi_z + b_hz)
    n   = tanh(gi_n + r * b_hn)
    h   = (1 - z) * n
Both matmuls are linear, so they fold into one K=128 contraction:
    gi = x @ W.T + bias,  W = w_ih @ w_emb,  bias = w_ih @ b_emb + b_ih.
1 - sigmoid(a) = sigmoid(-a), so the z block of W/bias is negated on the
host and the device computes
    h = sigmoid(gi_zneg) * tanh(gi_n + sigmoid(gi_r) * b_hn).

fp32 matmuls on trn2 run as two half-rate passes (~5x slower than 16-bit),
so the product is computed in fp16 with fp32 PSUM accumulation:
    x @ W ~= xh@Wh + xl@Wh                  (x = xh + xl, both fp16)
The only dropped term is x @ (W - fp16(W)) ~ 2^-11 relative.  The bias
rides inside the xl matmul: two x-channels of xl are set to 1.0 and that
term's private copy of Wh carries the fp16 hi/lo bias rows there (the two
channels are chosen to minimize the absmax of the dropped xl correction).
Using 16-bit operands also lets x be loaded pre-transposed via the 2-byte
DMA x-bar transpose (the matmul contracts over i, which must sit on SBUF
partitions), so no compute engine spends time transposing.

Per 128-token tile: 6 matmuls (2 per gate bank), one sigmoid over the
r/z PSUM bank pair (ACT), r*b_hn and + n-bank on DVE, tanh on ACT, and
the final (1-z)*n product on GPSIMD; tanh/h-mul are software-pipelined
one tile behind so the strict-FIFO ACT queue never delays the sigmoid
whose completion releases the r/z PSUM banks.

Distribution: pure data parallel over the batch dim, 8 NeuronCores,
16 batches (8192 tokens) per core.  Weights are replicated.
"""

import numpy as np
import ml_dtypes

B, S, I, Hd = 128, 512, 128, 512
G3 = 3 * Hd
N_CORES = 8
B_PER_CORE = B // N_CORES            # 16
TOK = B_PER_CORE * S                 # 8192 tokens per core
GROUP_TOK = 512                      # tokens per group (4 tiles of 128)
J = GROUP_TOK // 128                 # 4 tiles per group
N_GROUPS = TOK // GROUP_TOK          # 16 groups per core

F16 = np.float16

_compiled = {}


def _build_program():
    import concourse.bacc as bacc
    import concourse.tile as tile
    from concourse import mybir

    F32 = mybir.dt.float32
    HF = mybir.dt.float16
    AF = mybir.ActivationFunctionType
    ALU = mybir.AluOpType

    nc = bacc.Bacc()
    xh_p = nc.declare_dram_parameter("xh", [TOK, I], HF, isOutput=False)
    xl_p = nc.declare_dram_parameter("xl", [TOK, I], HF, isOutput=False)
    wh_p = nc.declare_dram_parameter("wh", [I, G3], HF, isOutput=False)
    wha_p = nc.declare_dram_parameter("wha", [I, G3], HF, isOutput=False)
    bhn_p = nc.declare_dram_parameter("bhn", [Hd], F32, isOutput=False)
    out_p = nc.declare_dram_parameter("out", [TOK, Hd], F32, isOutput=True)

    out_v = out_p.rearrange("(g j p) h -> g p j h", p=128, j=J)

    with tile.TileContext(nc) as tc:
        with (
            tc.tile_pool(name="const", bufs=1) as cpool,
            tc.tile_pool(name="xin", bufs=8) as xin_pool,
            tc.tile_pool(name="ps", bufs=2, space="PSUM") as ps_pool,
            tc.tile_pool(name="work", bufs=8) as wpool,
            tc.tile_pool(name="hout", bufs=6) as hpool,
        ):
            # first group's transposed x loads go ahead of the const DMAs:
            # everything shares the SP HWDGE FIFO and the first matmul
            # needs the transposes.
            xht0 = xin_pool.tile([I, GROUP_TOK], HF, tag="xht")
            nc.sync.dma_start(out=xht0, in_=xh_p[0:GROUP_TOK, :],
                              transpose=True)
            xlt0 = xin_pool.tile([I, GROUP_TOK], HF, tag="xlt")
            nc.sync.dma_start(out=xlt0, in_=xl_p[0:GROUP_TOK, :],
                              transpose=True)
            wh_sb = cpool.tile([I, G3], HF)
            nc.sync.dma_start(out=wh_sb, in_=wh_p[:])
            wha_sb = cpool.tile([I, G3], HF)
            nc.sync.dma_start(out=wha_sb, in_=wha_p[:])
            bhn_sb = cpool.tile([128, Hd], F32)
            nc.gpsimd.dma_start(out=bhn_sb,
                                in_=bhn_p[:].partition_broadcast(128))

            # Software pipeline: the tanh + h-mul of tile t run one tile
            # late, so on the strict-FIFO ACT queue sigma(t+1) is never
            # stuck behind tanh(t) (whose input arrives via DVE) and the
            # r/z PSUM banks release as early as possible.
            NT = N_GROUPS * J
            h_gs = {}
            rzs = {}
            t2s = {}

            def head(t):
                g, j = divmod(t, J)
                if j == 0:
                    if g == 0:
                        xht, xlt = xht0, xlt0
                    else:
                        t0 = g * GROUP_TOK
                        xht = xin_pool.tile([I, GROUP_TOK], HF, tag="xht")
                        nc.sync.dma_start(
                            out=xht, in_=xh_p[t0:t0 + GROUP_TOK, :],
                            transpose=True
                        )
                        xlt = xin_pool.tile([I, GROUP_TOK], HF, tag="xlt")
                        nc.sync.dma_start(
                            out=xlt, in_=xl_p[t0:t0 + GROUP_TOK, :],
                            transpose=True
                        )
                    h_gs[g] = hpool.tile([128, J, Hd], F32, name="h_g",
                                         tag="h_g")
                    head.x = (xht, xlt)
                xht, xlt = head.x
                xh_j = xht[:, j * 128:(j + 1) * 128]
                xl_j = xlt[:, j * 128:(j + 1) * 128]
                # r/z banks in one 2-bank tile that frees right after the
                # sigmoid; the n bank separate (it is consumed last, so
                # deeper buffering keeps the PE from stalling on it).
                rz_ps = ps_pool.tile([128, 2 * Hd], F32, tag="rz_ps")
                n_ps = ps_pool.tile([128, Hd], F32, tag="n_ps", bufs=4)
                for b in range(2):
                    s = slice(b * Hd, (b + 1) * Hd)
                    nc.tensor.matmul(rz_ps[:, s], lhsT=xl_j,
                                     rhs=wha_sb[:, s], start=True, stop=False)
                    nc.tensor.matmul(rz_ps[:, s], lhsT=xh_j,
                                     rhs=wh_sb[:, s], start=False, stop=True)
                # r and z' share one sigmoid over two adjacent PSUM banks
                rz = wpool.tile([128, 2 * Hd], F32, tag="rz", bufs=6)
                nc.scalar.activation(rz, rz_ps, AF.Sigmoid)
                rzs[t] = rz
                s = slice(2 * Hd, G3)
                nc.tensor.matmul(n_ps, lhsT=xl_j, rhs=wha_sb[:, s],
                                 start=True, stop=False)
                nc.tensor.matmul(n_ps, lhsT=xh_j, rhs=wh_sb[:, s],
                                 start=False, stop=True)
                t1 = wpool.tile([128, Hd], F32, tag="t1", bufs=6)
                nc.vector.tensor_tensor(
                    out=t1, in0=rz[:, 0:Hd], in1=bhn_sb, op=ALU.mult
                )
                t2 = wpool.tile([128, Hd], F32, tag="t2", bufs=6)
                nc.vector.tensor_tensor(out=t2, in0=t1, in1=n_ps, op=ALU.add)
                t2s[t] = t2

            def tail(t):
                g, j = divmod(t, J)
                nn_ = wpool.tile([128, Hd], F32, tag="nn", bufs=6)
                nc.scalar.activation(nn_, t2s.pop(t), AF.Tanh)
                rz = rzs.pop(t)
                nc.gpsimd.tensor_tensor(
                    out=h_gs[g][:, j, :], in0=rz[:, Hd:2 * Hd], in1=nn_,
                    op=ALU.mult,
                )
                if j == J - 1:
                    nc.sync.dma_start(out=out_v[g], in_=h_gs.pop(g))

            for t in range(NT):
                head(t)
                if t > 0:
                    tail(t - 1)
            tail(NT - 1)

    nc.finalize()
    return nc


def _split_f16(a):
    hi = a.astype(F16)
    lo = (a.astype(np.float64) - hi.astype(np.float64)).astype(F16)
    return hi, lo


def _prepare_consts(w_emb, b_emb, w_ih, b_ih, b_hh):
    # Fold the two linear layers (double precision for the fold itself).
    W = w_ih.astype(np.float64) @ w_emb.astype(np.float64)          # [3Hd, I]
    bias = w_ih.astype(np.float64) @ b_emb.astype(np.float64) + b_ih  # [3Hd]
    b_hr, b_hz, b_hn = b_hh[:Hd], b_hh[Hd:2 * Hd], b_hh[2 * Hd:]
    bias = bias.copy()
    bias[0:Hd] += b_hr
    bias[Hd:2 * Hd] += b_hz
    # 1 - sigmoid(a) = sigmoid(-a): negate the z block of W and bias.
    W[Hd:2 * Hd, :] *= -1.0
    bias[Hd:2 * Hd] *= -1.0
    wh = np.ascontiguousarray(W.T).astype(F16)                      # [I, 3Hd]
    bh, bl = _split_f16(bias)
    bhn = np.ascontiguousarray(b_hn).astype(np.float32)
    return wh, bh, bl, bhn


def _run(x, wh, bh, bl, bhn, trace=False):
    from concourse.bass_utils import run_bass_kernel_spmd

    if "nc" not in _compiled:
        _compiled["nc"] = _build_program()
    nc = _compiled["nc"]

    xh, xl = _split_f16(np.asarray(x, dtype=np.float32))
    xh = xh.reshape(N_CORES * TOK, I)
    xl = xl.reshape(N_CORES * TOK, I)
    # The bias rows ride inside the xl matmul: two channels of xl become
    # the constant 1.0 and that term's private copy of wh gets the bias
    # hi/lo rows there.  Pick the two channels that minimize the absmax
    # of the dropped xl-correction.
    p = np.abs(xl.astype(np.float32)).max(axis=0) * \
        np.abs(wh.astype(np.float32)).max(axis=1)
    c1, c2 = map(int, np.argsort(p)[:2])
    wha = wh.copy()
    wha[c1, :] = bh
    wha[c2, :] = bl
    xl = xl.copy()
    xl[:, c1] = 1.0
    xl[:, c2] = 1.0
    xh = xh.reshape(N_CORES, TOK, I)
    xl = xl.reshape(N_CORES, TOK, I)
    in_maps = [
        {"xh": xh[c], "xl": xl[c], "wh": wh, "wha": wha, "bhn": bhn}
        for c in range(N_CORES)
    ]
    res = run_bass_kernel_spmd(nc, in_maps, list(range(N_CORES)), trace=trace)
    full = np.stack([res.results[c]["out"] for c in range(N_CORES)], axis=0)
    full = full.reshape(B, S, Hd)
    return full, res


def kernel(x, w_emb, b_emb, w_ih, b_ih, b_hh):
    x = np.asarray(x, dtype=np.float32)
    consts = _prepare_consts(
        np.asarray(w_emb), np.asarray(b_emb), np.asarray(w_ih),
        np.asarray(b_ih), np.asarray(b_hh),
    )
    full, _ = _run(x, *consts, trace=False)
    H = np.ascontiguousarray(full[:, :-1, :])
    h_last = np.ascontiguousarray(full[:, -1, :][None])
    return (H, h_last)


# revision 27
# speedup vs baseline: 1.1324x; 1.0754x over previous
"""Trainium2 Bass kernel for nn_EncoderRNN (batched GRU-step encoder).

Math: the reference is
    emb = x @ w_emb.T + b_emb
    gi  = emb @ w_ih.T + b_ih
    r   = sigmoid(gi_r + b_hr); z = sigmoid(gi_z + b_hz)
    n   = tanh(gi_n + r * b_hn)
    h   = (1 - z) * n
Both matmuls are linear, so they fold into one K=128 contraction:
    gi = x @ W.T + bias,  W = w_ih @ w_emb,  bias = w_ih @ b_emb + b_ih.
1 - sigmoid(a) = sigmoid(-a), so the z block of W/bias is negated on the
host and the device computes
    h = sigmoid(gi_zneg) * tanh(gi_n + sigmoid(gi_r) * b_hn).

fp32 matmuls on trn2 run as two half-rate passes (~5x slower than 16-bit),
so the product is computed in fp16 with fp32 PSUM accumulation:
    x @ W ~= xh@Wh + xl@Wh                  (x = xh + xl, both fp16)
The only dropped term is x @ (W - fp16(W)) ~ 2^-11 relative.  The bias
rides inside the xl matmul: two x-channels of xl are set to 1.0 and that
term's private copy of Wh carries the fp16 hi/lo bias rows there (the two
channels are chosen to minimize the absmax of the dropped xl correction).
Using 16-bit operands also lets x be loaded pre-transposed via the 2-byte
DMA x-bar transpose (the matmul contracts over i, which must sit on SBUF
partitions), so no compute engine spends time transposing.

Per 128-token tile: 6 matmuls (2 per gate bank), one sigmoid over the
r/z PSUM bank pair (ACT), r*b_hn and + n-bank on DVE, tanh on ACT, and
the final (1-z)*n product on GPSIMD; tanh/h-mul are software-pipelined
one tile behind so the strict-FIFO ACT queue never delays the sigmoid
whose completion releases the r/z PSUM banks.

Distribution: pure data parallel over the batch dim, 8 NeuronCores,
16 batches (8192 tokens) per core.  Weights are replicated.
"""

import numpy as np
import ml_dtypes

B, S, I, Hd = 128, 512, 128, 512
G3 = 3 * Hd
N_CORES = 8
B_PER_CORE = B // N_CORES            # 16
TOK = B_PER_CORE * S                 # 8192 tokens per core
GROUP_TOK = 512                      # tokens per group (4 tiles of 128)
J = GROUP_TOK // 128                 # 4 tiles per group
N_GROUPS = TOK // GROUP_TOK          # 16 groups per core

F16 = np.float16

_compiled = {}


def _build_program():
    import concourse.bacc as bacc
    import concourse.tile as tile
    from concourse import mybir

    F32 = mybir.dt.float32
    HF = mybir.dt.float16
    AF = mybir.ActivationFunctionType
    ALU = mybir.AluOpType

    nc = bacc.Bacc()
    xh_p = nc.declare_dram_parameter("xh", [TOK, I], HF, isOutput=False)
    xl_p = nc.declare_dram_parameter("xl", [TOK, I], HF, isOutput=False)
    wh_p = nc.declare_dram_parameter("wh", [I, G3], HF, isOutput=False)
    wha_p = nc.declare_dram_parameter("wha", [I, G3], HF, isOutput=False)
    bhn_p = nc.declare_dram_parameter("bhn", [128, Hd], F32, isOutput=False)
    out_p = nc.declare_dram_parameter("out", [TOK, Hd], F32, isOutput=True)

    out_v = out_p.rearrange("(g j p) h -> g p j h", p=128, j=J)

    with tile.TileContext(nc) as tc:
        with (
            tc.tile_pool(name="const", bufs=1) as cpool,
            tc.tile_pool(name="xin", bufs=8) as xin_pool,
            tc.tile_pool(name="ps", bufs=2, space="PSUM") as ps_pool,
            tc.tile_pool(name="work", bufs=8) as wpool,
            tc.tile_pool(name="hout", bufs=6) as hpool,
        ):
            # first group's transposed x loads go ahead of the const DMAs:
            # everything shares the SP HWDGE FIFO and the first matmul
            # needs the transposes.
            xht0 = xin_pool.tile([I, GROUP_TOK], HF, tag="xht")
            nc.sync.dma_start(out=xht0, in_=xh_p[0:GROUP_TOK, :],
                              transpose=True)
            xlt0 = xin_pool.tile([I, GROUP_TOK], HF, tag="xlt")
            nc.sync.dma_start(out=xlt0, in_=xl_p[0:GROUP_TOK, :],
                              transpose=True)
            wh_sb = cpool.tile([I, G3], HF)
            nc.sync.dma_start(out=wh_sb, in_=wh_p[:])
            wha_sb = cpool.tile([I, G3], HF)
            nc.sync.dma_start(out=wha_sb, in_=wha_p[:])
            bhn_sb = cpool.tile([128, Hd], F32)
            nc.sync.dma_start(out=bhn_sb, in_=bhn_p[:])

            # Software pipeline: the tanh + h-mul of tile t run one tile
            # late, so on the strict-FIFO ACT queue sigma(t+1) is never
            # stuck behind tanh(t) (whose input arrives via DVE) and the
            # r/z PSUM banks release as early as possible.
            NT = N_GROUPS * J
            h_gs = {}
            rzs = {}
            t2s = {}

            def head(t):
                g, j = divmod(t, J)
                if j == 0:
                    if g == 0:
                        xht, xlt = xht0, xlt0
                    else:
                        t0 = g * GROUP_TOK
                        xht = xin_pool.tile([I, GROUP_TOK], HF, tag="xht")
                        nc.sync.dma_start(
                            out=xht, in_=xh_p[t0:t0 + GROUP_TOK, :],
                            transpose=True
                        )
                        xlt = xin_pool.tile([I, GROUP_TOK], HF, tag="xlt")
                        nc.sync.dma_start(
                            out=xlt, in_=xl_p[t0:t0 + GROUP_TOK, :],
                            transpose=True
                        )
                    h_gs[g] = hpool.tile([128, J, Hd], F32, name="h_g",
                                         tag="h_g")
                    head.x = (xht, xlt)
                xht, xlt = head.x
                xh_j = xht[:, j * 128:(j + 1) * 128]
                xl_j = xlt[:, j * 128:(j + 1) * 128]
                # r/z banks in one 2-bank tile that frees right after the
                # sigmoid; the n bank separate (it is consumed last, so
                # deeper buffering keeps the PE from stalling on it).
                rz_ps = ps_pool.tile([128, 2 * Hd], F32, tag="rz_ps")
                n_ps = ps_pool.tile([128, Hd], F32, tag="n_ps", bufs=4)
                for b in range(2):
                    s = slice(b * Hd, (b + 1) * Hd)
                    nc.tensor.matmul(rz_ps[:, s], lhsT=xh_j,
                                     rhs=wh_sb[:, s], start=True, stop=False)
                    nc.tensor.matmul(rz_ps[:, s], lhsT=xl_j,
                                     rhs=wha_sb[:, s], start=False, stop=True)
                # r and z' share one sigmoid over two adjacent PSUM banks
                rz = wpool.tile([128, 2 * Hd], F32, tag="rz", bufs=6)
                nc.scalar.activation(rz, rz_ps, AF.Sigmoid)
                rzs[t] = rz
                s = slice(2 * Hd, G3)
                nc.tensor.matmul(n_ps, lhsT=xh_j, rhs=wh_sb[:, s],
                                 start=True, stop=False)
                nc.tensor.matmul(n_ps, lhsT=xl_j, rhs=wha_sb[:, s],
                                 start=False, stop=True)
                t1 = wpool.tile([128, Hd], F32, tag="t1", bufs=6)
                nc.vector.tensor_tensor(
                    out=t1, in0=rz[:, 0:Hd], in1=bhn_sb, op=ALU.mult
                )
                t2 = wpool.tile([128, Hd], F32, tag="t2", bufs=6)
                nc.vector.tensor_tensor(out=t2, in0=t1, in1=n_ps, op=ALU.add)
                t2s[t] = t2

            def tail(t):
                g, j = divmod(t, J)
                nn_ = wpool.tile([128, Hd], F32, tag="nn", bufs=6)
                nc.scalar.activation(nn_, t2s.pop(t), AF.Tanh)
                rz = rzs.pop(t)
                nc.gpsimd.tensor_tensor(
                    out=h_gs[g][:, j, :], in0=rz[:, Hd:2 * Hd], in1=nn_,
                    op=ALU.mult,
                )
                if g == N_GROUPS - 1:
                    nc.sync.dma_start(out=out_v[g][:, j, :],
                                      in_=h_gs[g][:, j, :])
                    if j == J - 1:
                        h_gs.pop(g)
                elif j == J - 1:
                    nc.sync.dma_start(out=out_v[g], in_=h_gs.pop(g))

            for t in range(NT):
                head(t)
                if t > 0:
                    tail(t - 1)
            tail(NT - 1)

    nc.finalize()
    return nc


def _split_f16(a):
    hi = a.astype(F16)
    lo = (a.astype(np.float64) - hi.astype(np.float64)).astype(F16)
    return hi, lo


def _prepare_consts(w_emb, b_emb, w_ih, b_ih, b_hh):
    # Fold the two linear layers (double precision for the fold itself).
    W = w_ih.astype(np.float64) @ w_emb.astype(np.float64)          # [3Hd, I]
    bias = w_ih.astype(np.float64) @ b_emb.astype(np.float64) + b_ih  # [3Hd]
    b_hr, b_hz, b_hn = b_hh[:Hd], b_hh[Hd:2 * Hd], b_hh[2 * Hd:]
    bias = bias.copy()
    bias[0:Hd] += b_hr
    bias[Hd:2 * Hd] += b_hz
    # 1 - sigmoid(a) = sigmoid(-a): negate the z block of W and bias.
    W[Hd:2 * Hd, :] *= -1.0
    bias[Hd:2 * Hd] *= -1.0
    wh = np.ascontiguousarray(W.T).astype(F16)                      # [I, 3Hd]
    bh, bl = _split_f16(bias)
    bhn = np.broadcast_to(
        np.asarray(b_hn, dtype=np.float32)[None, :], (128, Hd)
    ).copy()
    return wh, bh, bl, bhn


def _run(x, wh, bh, bl, bhn, trace=False):
    from concourse.bass_utils import run_bass_kernel_spmd

    if "nc" not in _compiled:
        _compiled["nc"] = _build_program()
    nc = _compiled["nc"]

    xh, xl = _split_f16(np.asarray(x, dtype=np.float32))
    xh = xh.reshape(N_CORES * TOK, I)
    xl = xl.reshape(N_CORES * TOK, I)
    # The bias rows ride inside the xl matmul: two channels of xl become
    # the constant 1.0 and that term's private copy of wh gets the bias
    # hi/lo rows there.  Pick the two channels that minimize the absmax
    # of the dropped xl-correction.
    p = np.abs(xl.astype(np.float32)).max(axis=0) * \
        np.abs(wh.astype(np.float32)).max(axis=1)
    c1, c2 = map(int, np.argsort(p)[:2])
    wha = wh.copy()
    wha[c1, :] = bh
    wha[c2, :] = bl
    xl = xl.copy()
    xl[:, c1] = 1.0
    xl[:, c2] = 1.0
    xh = xh.reshape(N_CORES, TOK, I)
    xl = xl.reshape(N_CORES, TOK, I)
    in_maps = [
        {"xh": xh[c], "xl": xl[c], "wh": wh, "wha": wha, "bhn": bhn}
        for c in range(N_CORES)
    ]
    res = run_bass_kernel_spmd(nc, in_maps, list(range(N_CORES)), trace=trace)
    full = np.stack([res.results[c]["out"] for c in range(N_CORES)], axis=0)
    full = full.reshape(B, S, Hd)
    return full, res


def kernel(x, w_emb, b_emb, w_ih, b_ih, b_hh):
    x = np.asarray(x, dtype=np.float32)
    consts = _prepare_consts(
        np.asarray(w_emb), np.asarray(b_emb), np.asarray(w_ih),
        np.asarray(b_ih), np.asarray(b_hh),
    )
    full, _ = _run(x, *consts, trace=False)
    H = np.ascontiguousarray(full[:, :-1, :])
    h_last = np.ascontiguousarray(full[:, -1, :][None])
    return (H, h_last)
